# revision 1
# baseline (speedup 1.0000x reference)
"""MoLE (mixture-of-linear-experts) Trainium2 kernel.

Full-input contract: kernel(**inputs) takes the complete arrays from
setup_inputs() and returns the full (131072, 256) f32 output.

Math (reference):
    mixed[b] = sum_e expert_mixing_coeffs[b, e] * kernel[e]       # (B, IN, OUT)
    out      = concat_b( x_b @ mixed[b] ) + bias                  # (N, OUT)

Distribution: data-parallel over graphs. 8 graphs per NeuronCore, the
(E, IN, OUT) expert bank replicated to every core. No collectives.

Per-core device program (Bass/Tile):
  1. Mixing on TensorE: for each local graph g, accumulate 16 matmuls
     diag(c_ge).T @ kbank_e into PSUM (one N=512 f32 matmul per expert,
     rhs columns = (ihalf, out)).  diag(c_ge) is built on VectorE as
     identity * coeff (per-partition scalar from a broadcast tile).
     Result: mixed_g in SBUF as (128 part = i%128, 512 = (i//128, o)).
  2. Grouped GEMM on TensorE: out_tile(128 nodes, 256 o) accumulates
     lhsT = xT chunk (i-part, node-free) over the two 128-i halves with
     rhs = mixed_g half. x is handed to the device pre-transposed per
     graph so lhsT slices are natural SBUF slices.
  3. Bias add fused into the PSUM->SBUF copy (tensor_tensor add with a
     host-broadcast bias tile), 2 MiB output DMA per graph.

Host-side prep is layout-only (slicing/transposition, zero FLOPs).
"""

import sys

import numpy as np

sys.path.insert(0, "/opt/trn_rl_repo")

B = 64
E = 16
IN = 256
OUT = 256
NPG = 2048
N = B * NPG
NCORES = 8
GPC = B // NCORES          # graphs per core
NPC = GPC * NPG            # nodes per core
NT = NPG // 128            # node tiles per graph

_prog_cache = {}


def _build_program():
    if "nc" in _prog_cache:
        return _prog_cache["nc"]

    import concourse.bass as bass  # noqa: F401
    import concourse.mybir as mybir
    import concourse.tile as tile
    from concourse import bacc

    f32 = mybir.dt.float32
    mult = mybir.AluOpType.mult
    add = mybir.AluOpType.add

    nc = bacc.Bacc(
        "TRN2", target_bir_lowering=False, debug=False, num_devices=NCORES
    )

    # Per-core DRAM parameters (shard shapes / host-prepared layouts).
    # xt[g, p, ib*NPG + n] = x[g*NPG + n, ib*128 + p]
    xt = nc.dram_tensor("xt", [GPC, 128, 2 * NPG], f32, kind="ExternalInput").ap()
    # kb[p, (e*2 + ib)*OUT + o] = kernel[e, ib*128 + p, o]
    kb = nc.dram_tensor("kb", [128, E * 2 * OUT], f32, kind="ExternalInput").ap()
    # cb[p, g*E + e] = coeffs[core_g0 + g, e]  (broadcast over partitions)
    cb = nc.dram_tensor("cb", [128, 128], f32, kind="ExternalInput").ap()
    # idm = 128x128 identity
    idm = nc.dram_tensor("idm", [128, 128], f32, kind="ExternalInput").ap()
    # bb[p, o] = bias[o]  (broadcast over partitions)
    bb = nc.dram_tensor("bb", [128, OUT], f32, kind="ExternalInput").ap()
    # ot[g, p, nt*OUT + o] = out[g*NPG + nt*128 + p, o]
    ot = nc.dram_tensor("ot", [GPC, 128, NT * OUT], f32, kind="ExternalOutput").ap()

    with tile.TileContext(nc) as tc:
        with (
            tc.tile_pool(name="const", bufs=1) as constp,
            tc.tile_pool(name="kbank", bufs=1) as kbp,
            tc.tile_pool(name="diag", bufs=4) as diagp,
            tc.tile_pool(name="mix", bufs=1) as mixp,
            tc.tile_pool(name="xts", bufs=3) as xtp,
            tc.tile_pool(name="osb", bufs=2) as osbp,
            tc.tile_pool(name="pmix", bufs=2, space="PSUM") as pmixp,
            tc.tile_pool(name="pmain", bufs=6, space="PSUM") as pmainp,
        )        :
            cbt = constp.tile([128, 128], f32, tag="cbt")
            nc.sync.dma_start(out=cbt[:], in_=cb)
            idt = constp.tile([128, 128], f32, tag="idt")
            nc.sync.dma_start(out=idt[:], in_=idm)
            bt = constp.tile([128, OUT], f32, tag="bt")
            nc.sync.dma_start(out=bt[:], in_=bb)
            kbt = kbp.tile([128, E * 2 * OUT], f32, tag="kbt")
            nc.sync.dma_start(out=kbt[:], in_=kb)

            # x streams: kick off early, double/triple buffered by the pool.
            xts = {}

            def load_x(g):
                t = xtp.tile([128, 2 * NPG], f32)
                nc.sync.dma_start(out=t[:], in_=xt[g])
                xts[g] = t

            load_x(0)
            load_x(1)

            # ---- Phase 1: mix expert kernels per graph ----
            mixes = []
            for g in range(GPC):
                pm = pmixp.tile([128, 2 * OUT], f32)
                for e in range(E):
                    d = diagp.tile([128, 128], f32, tag="diag")
                    nc.vector.tensor_scalar(
                        d[:], idt[:], cbt[:, g * E + e : g * E + e + 1], None, mult
                    )
                    nc.tensor.matmul(
                        pm[:],
                        d[:],
                        kbt[:, e * 2 * OUT : (e + 1) * 2 * OUT],
                        start=(e == 0),
                        stop=(e == E - 1),
                    )
                mx = mixp.tile([128, 2 * OUT], f32, tag=f"mix{g}")
                nc.any.tensor_copy(mx[:], pm[:])
                mixes.append(mx)

            # ---- Phase 2: grouped GEMM + bias ----
            for g in range(GPC):
                if g + 2 < GPC:
                    load_x(g + 2)
                xg = xts.pop(g)
                og = osbp.tile([128, NT * OUT], f32)
                for nt in range(NT):
                    po = pmainp.tile([128, OUT], f32)
                    for ib in range(2):
                        nc.tensor.matmul(
                            po[:],
                            xg[:, ib * NPG + nt * 128 : ib * NPG + nt * 128 + 128],
                            mixes[g][:, ib * OUT : (ib + 1) * OUT],
                            start=(ib == 0),
                            stop=(ib == 1),
                        )
                    nc.any.tensor_tensor(
                        og[:, nt * OUT : (nt + 1) * OUT], po[:], bt[:], add
                    )
                nc.sync.dma_start(out=ot[g], in_=og[:])

    nc.compile()
    _prog_cache["nc"] = nc
    return nc


def _prep_core_inputs(inputs, expert_mixing_coeffs, kernel_bank, bias):
    """Host-side shard + layout prep (pure slicing/transposition)."""
    kb = (
        kernel_bank.reshape(E, 2, 128, OUT)
        .transpose(2, 0, 1, 3)
        .reshape(128, E * 2 * OUT)
    )
    kb = np.ascontiguousarray(kb)
    idm = np.eye(128, dtype=np.float32)
    bb = np.ascontiguousarray(np.broadcast_to(bias, (128, OUT)))

    in_maps = []
    for c in range(NCORES):
        xs = inputs[c * NPC : (c + 1) * NPC]
        # (g, n, ib, p) -> (g, p, ib, n)
        xtc = np.ascontiguousarray(
            xs.reshape(GPC, NPG, 2, 128).transpose(0, 3, 2, 1)
        ).reshape(GPC, 128, 2 * NPG)
        cc = expert_mixing_coeffs[c * GPC : (c + 1) * GPC].reshape(1, GPC * E)
        cbc = np.ascontiguousarray(np.broadcast_to(cc, (128, GPC * E)))
        in_maps.append({"xt": xtc, "kb": kb, "cb": cbc, "idm": idm, "bb": bb})
    return in_maps


def kernel(**inputs):
    from concourse.bass_utils import run_bass_kernel_spmd

    x = np.asarray(inputs["inputs"], dtype=np.float32)
    coeffs = np.asarray(inputs["expert_mixing_coeffs"], dtype=np.float32)
    kbank = np.asarray(inputs["kernel"], dtype=np.float32)
    bias = np.asarray(inputs["bias"], dtype=np.float32)

    nc = _build_program()
    in_maps = _prep_core_inputs(x, coeffs, kbank, bias)
    res = run_bass_kernel_spmd(nc, in_maps, list(range(NCORES)))

    out = np.empty((N, OUT), dtype=np.float32)
    for c in range(NCORES):
        oc = res.results[c]["ot"]  # (GPC, 128, NT*OUT)
        oc = oc.reshape(GPC, 128, NT, OUT).transpose(0, 2, 1, 3).reshape(NPC, OUT)
        out[c * NPC : (c + 1) * NPC] = oc
    return out
